# revision 5
# baseline (speedup 1.0000x reference)
"""NT-Xent (SimCLR) contrastive loss on 8 Trainium2 NeuronCores.

Strategy (data-parallel, hint-conformant):
  - Host shards N=4096 pairs across 8 cores: core c owns rows
    [c*512,(c+1)*512) of emb_i AND of emb_j (1024 rows of the stacked
    embedding matrix per core).
  - Phase A (on device, sharded): each core normalizes its own 1024 rows
    (ACT Square+accum -> Sqrt -> DVE reciprocal), casts to bf16, transposes
    its z-block via PE into [256,1024], and computes the positive-pair
    logits 2*z_i.z_j row-wise (DVE tensor_tensor_reduce).
  - AllGather (0.5 MB/rank, bf16) shares the transposed normalized blocks:
    every core ends with full z^T [256, 8192] bf16.
  - Phase B: 128 bf16 matmuls (N=512, fp32 PSUM) produce the 512x8192 logit
    rows; ACT Exp(scale=2, accum_out) fuses exp and the row-sum in one pass.
    Self-logit is exactly 2*|z|^2 ~= 2, so the denominator subtracts the
    constant e^2 via the Ln bias. Output: per-row loss terms [128,4]/core.
  - Host sums 8x512 values -> mean (the "psum the scalar" gather).
"""

import sys

if "/opt/trn_rl_repo" not in sys.path:
    sys.path.insert(0, "/opt/trn_rl_repo")

import numpy as np

import concourse.bass as bass
import concourse.mybir as mybir
import concourse.tile as tile
from concourse import bass_utils
from concourse.masks import make_identity

N_CORES = 8
N = 4096          # pairs
D = 256           # embedding dim
ROWS_PER_CORE = 2 * N // N_CORES      # 1024 rows of stacked E per core
OUT_ROWS = N // N_CORES               # 512 loss rows per core
INV_T = 2.0                           # 1 / temperature
E2_SELF = float(np.float32(np.exp(np.float32(2.0))))  # exp(2): self-logit term

FP32 = mybir.dt.float32
BF16 = mybir.dt.bfloat16


def _split_oversized_waits(nc, max_waits=1):
    """Walrus accepts at most one sync-wait per instruction; hoist extras
    onto preceding single-wait drains on the same engine (streams are FIFO
    per engine, so semantics are preserved)."""
    for bb in nc.main_func.blocks:
        new_list = []
        for ins in bb.instructions:
            si = ins.sync_info
            if si is not None and si.on_wait and len(si.on_wait) > max_waits:
                waits = list(si.on_wait)
                extra, keep = waits[:-max_waits], waits[-max_waits:]
                for gi, w in enumerate(extra):
                    d = mybir.InstDrain(name=f"{ins.name}-wsplit{gi}", engine=ins.engine)
                    d.sync_info = mybir.SyncInfo(on_wait=[w], on_update=[])
                    new_list.append(d)
                ins.sync_info = mybir.SyncInfo(on_wait=list(keep), on_update=list(si.on_update))
            new_list.append(ins)
        bb.instructions = new_list


def _build():
    nc = bass.Bass("TRN2", num_devices=N_CORES)
    e_own = nc.dram_tensor("e_own", [ROWS_PER_CORE, D], FP32, kind="ExternalInput")
    pp_out = nc.dram_tensor("pp_out", [128, 4], FP32, kind="ExternalOutput")

    e_chunks = e_own.ap().rearrange("(c p) d -> c p d", p=128)   # [8,128,256]

    with tile.TileContext(nc) as tc:
        with tc.tile_pool(name="dram", bufs=1, space="DRAM") as dram, \
             tc.tile_pool(name="persist", bufs=1) as persist, \
             tc.tile_pool(name="work", bufs=3) as work, \
             tc.tile_pool(name="small", bufs=4) as small:

            ag_in = dram.tile([2, 128, ROWS_PER_CORE], BF16)
            ag_out = dram.tile([N_CORES, 2, 128, ROWS_PER_CORE], BF16,
                               addr_space="Shared")

            ident = persist.tile([128, 128], BF16)
            make_identity(nc, ident)
            neg_e2 = persist.tile([128, 1], FP32)
            nc.vector.memset(neg_e2, -E2_SELF)

            zrows = persist.tile([128, 8, D], BF16)         # own z, row layout
            zT0 = persist.tile([128, ROWS_PER_CORE], BF16)  # own z^T, d 0:128
            zT1 = persist.tile([128, ROWS_PER_CORE], BF16)  # own z^T, d 128:256
            pos2 = persist.tile([128, 4], FP32)             # 2*z_i.z_j per row
            zt_full0 = persist.tile([128, 2 * N], BF16)     # gathered z^T d 0:128
            zt_full1 = persist.tile([128, 2 * N], BF16)     # gathered z^T d 128:256
            ppsb = persist.tile([128, 4], FP32)

            # ---------------- Phase A: normalize own rows, transpose ----------
            with tc.tile_pool(name="psumA", bufs=2, space="PSUM") as psumA:
                for ch in range(8):
                    et = work.tile([128, D], FP32, tag="et")
                    nc.sync.dma_start(et, e_chunks[ch])
                    sq = work.tile([128, D], BF16, tag="sq")
                    n2 = small.tile([128, 1], FP32, tag="n2")
                    nc.scalar.activation(sq, et, mybir.ActivationFunctionType.Square,
                                         accum_out=n2)
                    rn = small.tile([128, 1], FP32, tag="rn")
                    nc.scalar.activation(rn, n2, mybir.ActivationFunctionType.Sqrt)
                    inv = small.tile([128, 1], FP32, tag="inv")
                    nc.vector.reciprocal(inv, rn)
                    nc.vector.tensor_scalar_mul(zrows[:, ch, :], et, inv)
                    for k, zT in enumerate((zT0, zT1)):
                        pt = psumA.tile([128, 128], BF16, tag="pt")
                        nc.tensor.transpose(pt, zrows[:, ch, k * 128:(k + 1) * 128],
                                            ident)
                        nc.vector.tensor_copy(zT[:, ch * 128:(ch + 1) * 128], pt)

                # positive-pair raw dots: z_i[r].z_j[r]
                for m in range(4):
                    ttrs = work.tile([128, D], FP32, tag="ttrs")
                    nc.vector.tensor_mul(ttrs, zrows[:, m, :], zrows[:, m + 4, :])
                    nc.vector.tensor_reduce(pos2[:, m:m + 1], ttrs,
                                            axis=mybir.AxisListType.X,
                                            op=mybir.AluOpType.add)

                nc.sync.dma_start(ag_in[0], zT0)
                nc.sync.dma_start(ag_in[1], zT1)

                nc.gpsimd.collective_compute(
                    "AllGather", mybir.AluOpType.bypass,
                    replica_groups=[list(range(N_CORES))],
                    ins=[ag_in.opt()], outs=[ag_out.opt()])

                for c in range(N_CORES):
                    nc.sync.dma_start(
                        zt_full0[:, c * ROWS_PER_CORE:(c + 1) * ROWS_PER_CORE],
                        ag_out[c, 0])
                    nc.sync.dma_start(
                        zt_full1[:, c * ROWS_PER_CORE:(c + 1) * ROWS_PER_CORE],
                        ag_out[c, 1])

            # ---------------- Phase B: logits, exp-sum, loss rows -------------
            with tc.tile_pool(name="psumB", bufs=2, space="PSUM") as psumB, \
                 tc.tile_pool(name="esc", bufs=3) as escp, \
                 tc.tile_pool(name="stat", bufs=2) as statp:
                for m in range(4):
                    lhs0 = zT0[:, m * 128:(m + 1) * 128]
                    lhs1 = zT1[:, m * 128:(m + 1) * 128]
                    rs = statp.tile([128, 4], FP32, tag="rs")
                    for g in range(4):
                        S = psumB.tile([128, 2048], FP32, tag="S")
                        for cc in range(4):
                            sl = slice(cc * 512, (cc + 1) * 512)
                            col = g * 2048 + cc * 512
                            nc.tensor.matmul(S[:, sl], lhs0,
                                             zt_full0[:, col:col + 512],
                                             start=True, stop=False)
                            nc.tensor.matmul(S[:, sl], lhs1,
                                             zt_full1[:, col:col + 512],
                                             start=False, stop=True)
                        esc = escp.tile([128, 2048], BF16, tag="esc")
                        nc.scalar.activation(esc, S,
                                             mybir.ActivationFunctionType.Exp,
                                             scale=INV_T,
                                             accum_out=rs[:, g:g + 1])
                    rtot = statp.tile([128, 1], FP32, tag="rtot")
                    nc.vector.tensor_reduce(rtot, rs, axis=mybir.AxisListType.X,
                                            op=mybir.AluOpType.add)
                    logden = statp.tile([128, 1], FP32, tag="logden")
                    nc.scalar.activation(logden, rtot,
                                         mybir.ActivationFunctionType.Ln,
                                         bias=neg_e2[:, 0:1])
                    nc.vector.scalar_tensor_tensor(
                        out=ppsb[:, m:m + 1], in0=pos2[:, m:m + 1], scalar=-INV_T,
                        in1=logden,
                        op0=mybir.AluOpType.mult, op1=mybir.AluOpType.add)

                nc.sync.dma_start(pp_out.ap(), ppsb)

    _split_oversized_waits(nc)
    return nc


_NC_CACHE = None


def _get_nc():
    global _NC_CACHE
    if _NC_CACHE is None:
        _NC_CACHE = _build()
    return _NC_CACHE


def _make_in_maps(emb_i: np.ndarray, emb_j: np.ndarray):
    emb_i = np.ascontiguousarray(np.asarray(emb_i, dtype=np.float32))
    emb_j = np.ascontiguousarray(np.asarray(emb_j, dtype=np.float32))
    in_maps = []
    for c in range(N_CORES):
        sl = slice(c * OUT_ROWS, (c + 1) * OUT_ROWS)
        in_maps.append({"e_own": np.concatenate([emb_i[sl], emb_j[sl]], axis=0)})
    return in_maps


def kernel(emb_i: np.ndarray, emb_j: np.ndarray) -> np.ndarray:
    nc = _get_nc()
    in_maps = _make_in_maps(emb_i, emb_j)
    res = bass_utils.run_bass_kernel_spmd(nc, in_maps, core_ids=list(range(N_CORES)))
    total = 0.0
    for c in range(N_CORES):
        total += res.results[c]["pp_out"].astype(np.float64).sum()
    return np.float32(total / N)
